# revision 3
# baseline (speedup 1.0000x reference)
"""CrossNetMix (FuxiCTR MoE-routing) Trainium2 Bass kernel.

Math: the reference updates Xi = Xi + X0 * xw with xw of shape (B, 1), so
Xi is always a per-row scalar multiple of X0: Xi = c_b * X0[b].  With
precomputed per-row projections of X0 (g0 = X0@Wg^T, p0[l,e] =
sum_r (X0@U^T)(X0@V^T)), each layer reduces to a tiny per-row scalar
recurrence:

    gate_logits = c * g0[l] + bg[l]
    xw          = c^2 * sum_e(p0[l] * softmax(gate_logits))
    c          += xw

so the whole network is ONE fused matmul X0 @ W_all^T + an epilogue.

Product trick: with A = (U+V)/2, B = (U-V)/2 (packed host-side),
p0 = sum_r a^2 - sum_r b^2 where a = X0@A^T, b = X0@B^T.  The squares run
on the scalar (ACT) engine straight out of PSUM; only a segmented
reduce_sum runs on DVE.  This removes the PSUM->SBUF copy + elementwise
multiply of the naive u*v epilogue.

Sharding: data-parallel over batch across 8 NeuronCores; weights
replicated; no collectives.

Per-core schedule (Bc = 2048 rows = 16 m-panels of 128):
  - 2 phases x 8 panels.  X^T panel blocks (host-transposed, plain 2D
    DMAs) are phase-resident; the 6 A/B weight groups (512 cols each) +
    gate stream through a double-buffered pool, so the PE never waits on
    weights.
  - matmul dtype float32r: full-rate (1 cyc/row) fp32 PE mode.
  - group order per phase: A0 B0 G A1 B1 A2 B2.  The layer-l scalar
    recurrence for panel m is emitted right after B_l's epilogue of that
    panel, and the output panel (c * X0 row-major) is finalized during
    the last group -- the post-matmul serial tail is ~one panel.
"""

import os
import numpy as np

import concourse.bacc as bacc
import concourse.mybir as mybir
from concourse.tile import TileContext
from concourse.bass_utils import run_bass_kernel_spmd

# Problem constants (hardcoded per contest contract)
B, D, L, E, R = 16384, 2048, 3, 8, 64
N_CORES = 8
BC = B // N_CORES  # 2048 rows per core
P = 128
KT = D // P        # 16 contraction tiles
MT = BC // P       # 16 m-panels per core
MPP = 8            # m-panels per phase
MPB = 4            # m-panels per xt block (prefetch granule)
GW = E * R         # 512 = A/B group width (one layer, all experts)
N_UV = 2 * L       # 6 A/B groups
GATE = L * E       # 24 gate columns

_F32 = mybir.dt.float32


def build_nc(mm_dtype=mybir.dt.float32r, mt: int = MT, mpp: int = MPP,
             reps: int = 1):
    """Build the per-core Bass kernel. mt<MT builds a reduced-size kernel
    (for simulation); reps>1 wraps the body in a hardware loop (timing)."""
    nc = bacc.Bacc("TRN2", target_bir_lowering=False, debug=False,
                   num_devices=N_CORES)
    bc = mt * P
    mpp = min(mpp, mt)
    n_ph = (mt + mpp - 1) // mpp
    mpb = min(MPB, mpp)
    bpp = (mpp + mpb - 1) // mpb  # xt blocks per phase

    x0 = nc.dram_tensor("X0", [bc, D], _F32, kind="ExternalInput")
    xtb = nc.dram_tensor("XTB", [mt, P, KT * P], mm_dtype,
                         kind="ExternalInput")
    wab = nc.dram_tensor("WAB", [N_UV, P, KT * GW], mm_dtype,
                         kind="ExternalInput")
    wgd = nc.dram_tensor("WG", [P, KT * GATE], mm_dtype,
                         kind="ExternalInput")
    bgr = nc.dram_tensor("BG", [P, GATE], _F32, kind="ExternalInput")
    out = nc.dram_tensor("OUT", [bc, D], _F32, kind="ExternalOutput")

    # group sequence: (kind, layer); gate after B0 so the layer-0
    # recurrence can start, A/B of later layers after it.
    seq = [("A", 0), ("B", 0), ("G", 0), ("A", 1), ("B", 1),
           ("A", 2), ("B", 2)]

    with TileContext(nc) as tc:
        with (
            tc.tile_pool(name="xt_p", bufs=3) as xt_pool,
            tc.tile_pool(name="w_p", bufs=2) as w_pool,
            tc.tile_pool(name="wg_p", bufs=1) as wg_pool,
            tc.tile_pool(name="pg_p", bufs=1) as pg_pool,
            tc.tile_pool(name="sq_p", bufs=3) as sq_pool,
            tc.tile_pool(name="sm_p", bufs=2) as sm_pool,
            tc.tile_pool(name="xp_p", bufs=2) as xp_pool,
            tc.tile_pool(name="ps_p", bufs=6, space="PSUM") as ps_pool,
        ):
            # --- persistent tiles ---
            wg_sb = wg_pool.tile([P, KT * GATE], mm_dtype, tag="wg")
            bg_sb = wg_pool.tile([P, GATE], _F32, tag="bg")
            # per panel m: [pA0 pB0 pA1 pB1 pA2 pB2 (8 each) | g0 (24)]
            pg_sb = pg_pool.tile([P, mt * 72], _F32, tag="pg")
            c_sb = wg_pool.tile([P, mt], _F32, tag="c")

            nc.scalar.dma_start(out=wg_sb[:], in_=wgd[:])
            nc.scalar.dma_start(out=bg_sb[:], in_=bgr[:])

            def recurrence(m, l):
                c_m = c_sb[:, m : m + 1]
                base = m * 72
                pa = pg_sb[:, base + 2 * l * 8 : base + 2 * l * 8 + 8]
                pb = pg_sb[:, base + (2 * l + 1) * 8 : base + (2 * l + 2) * 8]
                g0l = pg_sb[:, base + 48 + l * E : base + 48 + (l + 1) * E]
                d = sm_pool.tile([P, E], _F32, tag="d")
                t = sm_pool.tile([P, E], _F32, tag="t")
                et = sm_pool.tile([P, E], _F32, tag="et")
                nmx = sm_pool.tile([P, 1], _F32, tag="nmx")
                s1 = sm_pool.tile([P, 1], _F32, tag="s1")
                s2 = sm_pool.tile([P, 1], _F32, tag="s2")
                rcp = sm_pool.tile([P, 1], _F32, tag="rcp")
                e1 = sm_pool.tile([P, 1], _F32, tag="e1")
                # d = pA - pB  (= p0[l])
                nc.vector.tensor_tensor(d[:], pa, pb,
                                        op=mybir.AluOpType.subtract)
                # t = c * g0[l] + bg[l]
                nc.vector.scalar_tensor_tensor(
                    t[:], g0l, c_m, bg_sb[:, l * E : (l + 1) * E],
                    op0=mybir.AluOpType.mult, op1=mybir.AluOpType.add,
                )
                # nmx = -max_e t
                nc.vector.tensor_reduce(
                    nmx[:], t[:], axis=mybir.AxisListType.X,
                    op=mybir.AluOpType.max, negate=True,
                )
                # et = exp(t - max); s2 = sum_e et
                nc.scalar.activation(
                    et[:], t[:], mybir.ActivationFunctionType.Exp,
                    bias=nmx[:], scale=1.0, accum_out=s2[:],
                )
                # s1 = sum_e d * et
                nc.vector.scalar_tensor_tensor(
                    t[:], d[:], 1.0, et[:],
                    op0=mybir.AluOpType.mult, op1=mybir.AluOpType.mult,
                    accum_out=s1[:],
                )
                nc.vector.reciprocal(rcp[:], s2[:])
                # e1 = s1 * rcp * c ; c += e1 * c
                nc.vector.scalar_tensor_tensor(
                    e1[:], s1[:], rcp[:], c_m,
                    op0=mybir.AluOpType.mult, op1=mybir.AluOpType.mult,
                )
                nc.vector.scalar_tensor_tensor(
                    c_m, e1[:], c_m, c_m,
                    op0=mybir.AluOpType.mult, op1=mybir.AluOpType.add,
                )

            def body(_iv=None):
                nc.vector.memset(c_sb[:], 1.0)
                for ph in range(n_ph):
                    lo = ph * mpp
                    npan = min(mpp, mt - lo)
                    # phase-resident X^T blocks (plain 2D DMAs per panel)
                    blks = []
                    for b in range(bpp):
                        xt_sb = xt_pool.tile([P, mpb * KT * P], mm_dtype,
                                             tag="xt")
                        blks.append(xt_sb)
                        for j in range(mpb):
                            mloc = b * mpb + j
                            if mloc >= npan:
                                break
                            nc.sync.dma_start(
                                out=xt_sb[:, j * KT * P : (j + 1) * KT * P],
                                in_=xtb[lo + mloc],
                            )

                    def xt_panel(mloc, k):
                        t = blks[mloc // mpb]
                        j = mloc % mpb
                        base = (j * KT + k) * P
                        return t[:, base : base + P]

                    for kind, l in seq:
                        if kind == "G":
                            w_sb, ncols = wg_sb, GATE
                        else:
                            g = 2 * l + (kind == "B")
                            w_sb = w_pool.tile([P, KT * GW], mm_dtype,
                                               tag="w")
                            nc.scalar.dma_start(out=w_sb[:], in_=wab[g])
                            ncols = GW
                        last = kind == "B" and l == L - 1
                        for mloc in range(npan):
                            m = lo + mloc
                            xp = None
                            if last:
                                # prefetch X0 panel for the finalize
                                xp = xp_pool.tile([P, D], _F32, tag="xp")
                                nc.sync.dma_start(
                                    out=xp[:],
                                    in_=x0[m * P : (m + 1) * P, :],
                                )
                            ps = ps_pool.tile([P, GW], _F32, tag="ps")
                            for k in range(KT):
                                nc.tensor.matmul(
                                    ps[:, :ncols],
                                    xt_panel(mloc, k),
                                    w_sb[:, k * ncols : (k + 1) * ncols],
                                    start=(k == 0),
                                    stop=(k == KT - 1),
                                )
                            base = m * 72
                            if kind == "G":
                                nc.vector.tensor_copy(
                                    pg_sb[:, base + 48 : base + 72],
                                    ps[:, :GATE],
                                )
                                recurrence(m, 0)
                            else:
                                sq = sq_pool.tile([P, GW], _F32, tag="sq")
                                nc.scalar.activation(
                                    sq[:], ps[:],
                                    mybir.ActivationFunctionType.Square,
                                )
                                slot = base + (2 * l + (kind == "B")) * 8
                                nc.vector.reduce_sum(
                                    pg_sb[:, slot : slot + 8],
                                    sq[:].rearrange("p (e r) -> p e r", e=E),
                                    axis=mybir.AxisListType.X,
                                )
                                if kind == "B" and l >= 1:
                                    recurrence(m, l)
                                if last:
                                    # out panel = c * X0 panel
                                    c_m = c_sb[:, m : m + 1]
                                    nc.vector.tensor_scalar_mul(
                                        xp[:], xp[:], c_m
                                    )
                                    nc.sync.dma_start(
                                        out=out[m * P : (m + 1) * P, :],
                                        in_=xp[:],
                                    )

            if reps == 1:
                body()
            else:
                with tc.For_i(0, reps, 1) as iv:
                    body(iv)

    nc.compile()
    return nc


def pack_weights(U, V, Wg):
    """Host-side packing: A/B groups + gate, laid out so every device DMA
    is a plain contiguous [128, N] transfer."""
    A = (U + V) * 0.5  # (L, E, R, D)
    Bm = (U - V) * 0.5
    allw = np.empty((N_UV, GW, D), np.float32)
    for l in range(L):
        allw[2 * l] = A[l].reshape(GW, D)
        allw[2 * l + 1] = Bm[l].reshape(GW, D)
    # [g, c, k, p] -> [g, p, k, c]
    wab = np.ascontiguousarray(
        allw.reshape(N_UV, GW, KT, P).transpose(0, 3, 2, 1)
    ).reshape(N_UV, P, KT * GW)
    wg = np.ascontiguousarray(
        Wg.reshape(GATE, KT, P).transpose(2, 1, 0)
    ).reshape(P, KT * GATE)
    return wab, wg


def pack_xtb(x0_shard, mt=MT):
    """[bc, D] -> [mt, P(d-within-k), KT*P(b-within-panel)] blocked
    transpose so each panel is one contiguous [128, 2048] DMA."""
    return np.ascontiguousarray(
        x0_shard.reshape(mt, P, KT, P).transpose(0, 3, 2, 1)
    ).reshape(mt, P, KT * P)


def make_in_maps(X0, U, V, Wg, bg):
    X0 = np.ascontiguousarray(np.asarray(X0, dtype=np.float32))
    wab, wg = pack_weights(
        np.asarray(U, np.float32), np.asarray(V, np.float32),
        np.asarray(Wg, np.float32)
    )
    bg_rep = np.ascontiguousarray(
        np.broadcast_to(np.asarray(bg, np.float32).reshape(1, GATE),
                        (P, GATE))
    )
    in_maps = []
    for c in range(N_CORES):
        sh = X0[c * BC : (c + 1) * BC]
        in_maps.append(
            {
                "X0": sh,
                "XTB": pack_xtb(sh),
                "WAB": wab,
                "WG": wg,
                "BG": bg_rep,
            }
        )
    return in_maps


_CACHE = {}


def _get_runner(mm_dtype_name: str):
    key = mm_dtype_name
    if key not in _CACHE:
        _CACHE[key] = build_nc(getattr(mybir.dt, mm_dtype_name))
    return _CACHE[key]


def kernel(X0, U, V, Wg, bg):
    in_maps = make_in_maps(X0, U, V, Wg, bg)
    mm_dtype_name = os.environ.get("KERNEL_MM_DTYPE", "float32r")
    nc = _get_runner(mm_dtype_name)
    res = run_bass_kernel_spmd(nc, in_maps, list(range(N_CORES)))
    return np.concatenate(
        [res.results[c]["OUT"] for c in range(N_CORES)], axis=0
    )


# revision 11
# speedup vs baseline: 1.4222x; 1.4222x over previous
"""CrossNetMix (FuxiCTR MoE-routing) Trainium2 Bass kernel.

Math: the reference updates Xi = Xi + X0 * xw with xw of shape (B, 1), so
Xi is always a per-row scalar multiple of X0: Xi = c_b * X0[b].  With
precomputed per-row projections of X0 (g0 = X0@Wg^T, p0[l,e] =
sum_r (X0@U^T)(X0@V^T)), each layer reduces to a tiny per-row scalar
recurrence:

    gate_logits = c * g0[l] + bg[l]
    xw          = c^2 * sum_e(p0[l] * softmax(gate_logits))
    c          += xw

so the whole network is ONE fused matmul X0 @ W_all^T + an epilogue.

Product trick: with A = (U+V)/2, B = (U-V)/2 (packed host-side),
p0 = sum_r a^2 - sum_r b^2 where a = X0@A^T, b = X0@B^T.  The squares run
on the scalar (ACT) engine straight out of PSUM; only a segmented
reduce_sum runs on DVE.  This removes the PSUM->SBUF copy + elementwise
multiply of the naive u*v epilogue.

Sharding: data-parallel over batch across 8 NeuronCores; weights
replicated; no collectives.

Per-core schedule (Bc = 2048 rows = 16 m-panels of 128):
  - 2 phases x 8 panels.  X^T panel blocks (host-transposed, plain 2D
    DMAs) are phase-resident; the 6 A/B weight groups (512 cols each) +
    gate stream through a double-buffered pool, so the PE never waits on
    weights.
  - matmul dtype float32r: full-rate (1 cyc/row) fp32 PE mode.
  - group order per phase: A0 B0 G A1 B1 A2 B2.  The layer-l scalar
    recurrence for panel m is emitted right after B_l's epilogue of that
    panel, and the output panel (c * X0 row-major) is finalized during
    the last group -- the post-matmul serial tail is ~one panel.
"""

import os
import numpy as np

import concourse.bacc as bacc
import concourse.mybir as mybir
from concourse.tile import TileContext
from concourse.bass_utils import run_bass_kernel_spmd

# Problem constants (hardcoded per contest contract)
B, D, L, E, R = 16384, 2048, 3, 8, 64
N_CORES = 8
BC = B // N_CORES  # 2048 rows per core
P = 128
KT = D // P        # 16 contraction tiles
MT = BC // P       # 16 m-panels per core
MPP = 8            # m-panels per phase
MPB = 4            # m-panels per xt block (prefetch granule)
GW = E * R         # 512 = A/B group width (one layer, all experts)
N_UV = 2 * L       # 6 A/B groups
GATE = L * E       # 24 gate columns

_F32 = mybir.dt.float32


def build_nc(mm_dtype=mybir.dt.float32r, mt: int = MT, mpp: int = MPP,
             reps: int = 1):
    """Build the per-core Bass kernel. mt<MT builds a reduced-size kernel
    (for simulation); reps>1 wraps the body in a hardware loop (timing).
    KSTAGE env: 1=matmuls only, 2=+epilogue, 3=full (default)."""
    stage = int(os.environ.get("KSTAGE", "3"))
    nc = bacc.Bacc("TRN2", target_bir_lowering=False, debug=False,
                   num_devices=N_CORES)
    bc = mt * P
    mpp = min(mpp, mt)
    n_ph = (mt + mpp - 1) // mpp
    mpb = min(MPB, mpp)
    bpp = (mpp + mpb - 1) // mpb  # xt blocks per phase

    x0 = nc.dram_tensor("X0", [bc, D], _F32, kind="ExternalInput")
    xtb = nc.dram_tensor("XTB", [mt, P, KT * P], mm_dtype,
                         kind="ExternalInput")
    wab = nc.dram_tensor("WAB", [N_UV, P, KT * GW], mm_dtype,
                         kind="ExternalInput")
    wgd = nc.dram_tensor("WG", [P, KT * GATE], mm_dtype,
                         kind="ExternalInput")
    bgr = nc.dram_tensor("BG", [P, GATE], _F32, kind="ExternalInput")
    out = nc.dram_tensor("OUT", [bc, D], _F32, kind="ExternalOutput")

    # group sequence: (kind, layer); gate after B0 so the layer-0
    # recurrence can start, A/B of later layers after it.
    seq = [("A", 0), ("B", 0), ("G", 0), ("A", 1), ("B", 1),
           ("A", 2), ("B", 2)]

    with TileContext(nc) as tc:
        with (
            tc.tile_pool(name="xt_p", bufs=3) as xt_pool,
            tc.tile_pool(name="w_p", bufs=2) as w_pool,
            tc.tile_pool(name="wg_p", bufs=1) as wg_pool,
            tc.tile_pool(name="pg_p", bufs=1) as pg_pool,
            tc.tile_pool(name="sq_p", bufs=3) as sq_pool,
            tc.tile_pool(name="sm_p", bufs=2) as sm_pool,
            tc.tile_pool(name="xp_p", bufs=2) as xp_pool,
            tc.tile_pool(name="ps_p", bufs=int(os.environ.get("PSB", "2")),
                         space="PSUM") as ps_pool,
        ):
            # --- persistent tiles ---
            wg_sb = wg_pool.tile([P, KT * GATE], mm_dtype, tag="wg")
            bg_sb = wg_pool.tile([P, GATE], _F32, tag="bg")
            # per panel m: [pA0 pB0 pA1 pB1 pA2 pB2 (8 each) | g0 (24)]
            pg_sb = pg_pool.tile([P, mt * 72], _F32, tag="pg")
            c_sb = wg_pool.tile([P, mt], _F32, tag="c")

            nc.scalar.dma_start(out=wg_sb[:], in_=wgd[:])
            nc.scalar.dma_start(out=bg_sb[:], in_=bgr[:])

            def recurrence(m, l):
                c_m = c_sb[:, m : m + 1]
                base = m * 72
                pa = pg_sb[:, base + 2 * l * 8 : base + 2 * l * 8 + 8]
                pb = pg_sb[:, base + (2 * l + 1) * 8 : base + (2 * l + 2) * 8]
                g0l = pg_sb[:, base + 48 + l * E : base + 48 + (l + 1) * E]
                d = sm_pool.tile([P, E], _F32, tag="d")
                t = sm_pool.tile([P, E], _F32, tag="t")
                et = sm_pool.tile([P, E], _F32, tag="et")
                nmx = sm_pool.tile([P, 1], _F32, tag="nmx")
                s1 = sm_pool.tile([P, 1], _F32, tag="s1")
                s2 = sm_pool.tile([P, 1], _F32, tag="s2")
                rcp = sm_pool.tile([P, 1], _F32, tag="rcp")
                e1 = sm_pool.tile([P, 1], _F32, tag="e1")
                # d = pA - pB  (= p0[l])
                nc.vector.tensor_tensor(d[:], pa, pb,
                                        op=mybir.AluOpType.subtract)
                # t = c * g0[l] + bg[l]
                nc.vector.scalar_tensor_tensor(
                    t[:], g0l, c_m, bg_sb[:, l * E : (l + 1) * E],
                    op0=mybir.AluOpType.mult, op1=mybir.AluOpType.add,
                )
                # nmx = -max_e t
                nc.vector.tensor_reduce(
                    nmx[:], t[:], axis=mybir.AxisListType.X,
                    op=mybir.AluOpType.max, negate=True,
                )
                # et = exp(t - max); s2 = sum_e et
                nc.scalar.activation(
                    et[:], t[:], mybir.ActivationFunctionType.Exp,
                    bias=nmx[:], scale=1.0, accum_out=s2[:],
                )
                # s1 = sum_e d * et
                nc.vector.scalar_tensor_tensor(
                    t[:], d[:], 1.0, et[:],
                    op0=mybir.AluOpType.mult, op1=mybir.AluOpType.mult,
                    accum_out=s1[:],
                )
                nc.vector.reciprocal(rcp[:], s2[:])
                # e1 = s1 * rcp * c ; c += e1 * c
                nc.vector.scalar_tensor_tensor(
                    e1[:], s1[:], rcp[:], c_m,
                    op0=mybir.AluOpType.mult, op1=mybir.AluOpType.mult,
                )
                nc.vector.scalar_tensor_tensor(
                    c_m, e1[:], c_m, c_m,
                    op0=mybir.AluOpType.mult, op1=mybir.AluOpType.add,
                )

            def body(_iv=None):
                xp_tiles = {}
                nc.vector.memset(c_sb[:], 1.0)
                for ph in range(n_ph):
                    lo = ph * mpp
                    npan = min(mpp, mt - lo)
                    # phase-resident X^T blocks (plain 2D DMAs per panel)
                    blks = []
                    for b in range(bpp):
                        xt_sb = xt_pool.tile([P, mpb * KT * P], mm_dtype,
                                             tag="xt")
                        blks.append(xt_sb)
                        for j in range(mpb):
                            mloc = b * mpb + j
                            if mloc >= npan:
                                break
                            nc.gpsimd.dma_start(
                                out=xt_sb[:, j * KT * P : (j + 1) * KT * P],
                                in_=xtb[lo + mloc],
                            )

                    def xt_panel(mloc, k):
                        t = blks[mloc // mpb]
                        j = mloc % mpb
                        base = (j * KT + k) * P
                        return t[:, base : base + P]

                    for kind, l in seq:
                        if kind == "G":
                            w_sb, ncols = wg_sb, GATE
                        else:
                            g = 2 * l + (kind == "B")
                            w_sb = w_pool.tile([P, KT * GW], mm_dtype,
                                               tag="w")
                            nc.scalar.dma_start(out=w_sb[:], in_=wab[g])
                            ncols = GW
                        last = kind == "B" and l == L - 1
                        for mloc in range(npan):
                            m = lo + mloc
                            if stage >= 3 and kind == "A" and l == L - 1:
                                # prefetch X0 panel for the finalize, one
                                # group ahead of its use in B2
                                xp = xp_pool.tile([P, D], _F32, tag="xp")
                                xp_tiles[m] = xp
                                nc.gpsimd.dma_start(
                                    out=xp[:],
                                    in_=x0[m * P : (m + 1) * P, :],
                                )
                            ps = ps_pool.tile([P, GW], _F32, tag="ps")
                            for k in range(KT):
                                nc.tensor.matmul(
                                    ps[:, :ncols],
                                    xt_panel(mloc, k),
                                    w_sb[:, k * ncols : (k + 1) * ncols],
                                    start=(k == 0),
                                    stop=(k == KT - 1),
                                )
                            base = m * 72
                            if kind == "G":
                                nc.vector.tensor_copy(
                                    pg_sb[:, base + 48 : base + 72],
                                    ps[:, :GATE],
                                )
                                recurrence(m, 0)
                            else:
                                sq = sq_pool.tile([P, GW], _F32, tag="sq")
                                nc.scalar.activation(
                                    sq[:], ps[:],
                                    mybir.ActivationFunctionType.Square,
                                )
                                slot = base + (2 * l + (kind == "B")) * 8
                                nc.vector.reduce_sum(
                                    pg_sb[:, slot : slot + 8],
                                    sq[:].rearrange("p (e r) -> p e r", e=E),
                                    axis=mybir.AxisListType.X,
                                )
                                if kind == "B" and l >= 1:
                                    recurrence(m, l)
                                if last:
                                    # out panel = c * X0 panel
                                    xp = xp_tiles.pop(m)
                                    c_m = c_sb[:, m : m + 1]
                                    nc.vector.tensor_scalar_mul(
                                        xp[:], xp[:], c_m
                                    )
                                    nc.sync.dma_start(
                                        out=out[m * P : (m + 1) * P, :],
                                        in_=xp[:],
                                    )

            if reps == 1:
                body()
            else:
                with tc.For_i(0, reps, 1) as iv:
                    body(iv)

    nc.compile()
    return nc


def pack_weights(U, V, Wg):
    """Host-side packing: A/B groups + gate, laid out so every device DMA
    is a plain contiguous [128, N] transfer."""
    A = (U + V) * 0.5  # (L, E, R, D)
    Bm = (U - V) * 0.5
    allw = np.empty((N_UV, GW, D), np.float32)
    for l in range(L):
        allw[2 * l] = A[l].reshape(GW, D)
        allw[2 * l + 1] = Bm[l].reshape(GW, D)
    # [g, c, k, p] -> [g, p, k, c]
    wab = np.ascontiguousarray(
        allw.reshape(N_UV, GW, KT, P).transpose(0, 3, 2, 1)
    ).reshape(N_UV, P, KT * GW)
    wg = np.ascontiguousarray(
        Wg.reshape(GATE, KT, P).transpose(2, 1, 0)
    ).reshape(P, KT * GATE)
    return wab, wg


def pack_xtb(x0_shard, mt=MT):
    """[bc, D] -> [mt, P(d-within-k), KT*P(b-within-panel)] blocked
    transpose so each panel is one contiguous [128, 2048] DMA."""
    return np.ascontiguousarray(
        x0_shard.reshape(mt, P, KT, P).transpose(0, 3, 2, 1)
    ).reshape(mt, P, KT * P)


def make_in_maps(X0, U, V, Wg, bg):
    X0 = np.ascontiguousarray(np.asarray(X0, dtype=np.float32))
    wab, wg = pack_weights(
        np.asarray(U, np.float32), np.asarray(V, np.float32),
        np.asarray(Wg, np.float32)
    )
    bg_rep = np.ascontiguousarray(
        np.broadcast_to(np.asarray(bg, np.float32).reshape(1, GATE),
                        (P, GATE))
    )
    in_maps = []
    for c in range(N_CORES):
        sh = X0[c * BC : (c + 1) * BC]
        in_maps.append(
            {
                "X0": sh,
                "XTB": pack_xtb(sh),
                "WAB": wab,
                "WG": wg,
                "BG": bg_rep,
            }
        )
    return in_maps


_CACHE = {}


def _get_runner(mm_dtype_name: str):
    key = mm_dtype_name
    if key not in _CACHE:
        _CACHE[key] = build_nc(getattr(mybir.dt, mm_dtype_name))
    return _CACHE[key]


def kernel(X0, U, V, Wg, bg):
    in_maps = make_in_maps(X0, U, V, Wg, bg)
    mm_dtype_name = os.environ.get("KERNEL_MM_DTYPE", "float32r")
    nc = _get_runner(mm_dtype_name)
    res = run_bass_kernel_spmd(nc, in_maps, list(range(N_CORES)))
    return np.concatenate(
        [res.results[c]["OUT"] for c in range(N_CORES)], axis=0
    )
